# revision 1
# baseline (speedup 1.0000x reference)
"""Allegro GNN on 8 TRN2 NeuronCores — Bass/Tile kernel, v2.

Sharding: nodes partitioned across cores (256/core); edges routed to the core
owning their *sender* node → scatter-sum and gather-back are core-local (zero
collectives).

v2 changes vs baseline:
  - cst (CG x Wv fold) precomputed on host (was 64 device MMs + copies)
  - tp0 split out of the cst contraction (cst cols 304 -> 240, 4 MMs/m not 6)
    using the exact diagonality of the (l,l,0) couplings
  - L2 tensor-product done feature-major (strided gather AP + 0/1 contraction)
    replacing the transpose-heavy per-tile block
  - persistent env broadcast tile (esb) instead of per-use rebuilds
  - m-loop software-pipelined; pr broadcast optionally emitted as
    transpose-mode matmul with bf16 PSUM out (2x DVE mode for the product)
"""
import math
import sys

import numpy as np

sys.path.insert(0, "/opt/trn_rl_repo")

import concourse.bacc as bacc  # noqa: E402
import concourse.bass as bass  # noqa: E402
import concourse.mybir as mybir  # noqa: E402
from concourse import tile  # noqa: E402
from concourse.bass_utils import run_bass_kernel_spmd  # noqa: E402
import ml_dtypes  # noqa: E402

F32 = mybir.dt.float32
BF16 = mybir.dt.bfloat16
BF = ml_dtypes.bfloat16
AL = mybir.AluOpType
AF = mybir.ActivationFunctionType

E, NNODE = 32768, 2048
NUM_SPECIES, EMB = 100, 32
MUL, HIDDEN, N_RBF, LMAX = 16, 256, 8, 3
N_CORES = 8
NPC = NNODE // N_CORES          # nodes per core
SLSTART = {0: 0, 1: 1, 2: 4, 3: 9}
SLICES = {0: (0, 1), 1: (1, 4), 2: (4, 9), 3: (9, 16)}

# pr broadcast via transpose-mode matmul -> bf16 PSUM -> 2x DVE TT
PR_TRANSPOSE = False
# number of m-channels whose pr evacuation goes через ACT (rest: direct TT)
ACT_EVAC = 6

# ---------------------------------------------------------------- CG tensors


def _cg(j1, m1, j2, m2, j3, m3):
    if m1 + m2 != m3:
        return 0.0
    f = math.factorial
    pre = math.sqrt((2*j3+1) * f(j1+j2-j3) * f(j1-j2+j3) * f(-j1+j2+j3) / f(j1+j2+j3+1))
    pre *= math.sqrt(f(j3+m3)*f(j3-m3)*f(j1-m1)*f(j1+m1)*f(j2-m2)*f(j2+m2))
    s = 0.0
    kmin = max(0, j2 - j3 - m1, j1 - j3 + m2)
    kmax = min(j1 + j2 - j3, j1 - m1, j2 + m2)
    for k in range(kmin, kmax + 1):
        s += (-1)**k / (f(k)*f(j1+j2-j3-k)*f(j1-m1-k)*f(j2+m2-k)*f(j3-j2+m1+k)*f(j3-j1-m2+k))
    return pre * s


def _umat(l):
    U = np.zeros((2*l+1, 2*l+1), dtype=complex)
    U[l, l] = 1.0
    s2 = 1.0 / math.sqrt(2.0)
    for m in range(1, l + 1):
        U[l+m, l-m] = s2
        U[l+m, l+m] = (-1)**m * s2
        U[l-m, l-m] = 1j * s2
        U[l-m, l+m] = -1j * (-1)**m * s2
    return U


def _real_coupling(l1, l2, l3):
    C = np.zeros((2*l1+1, 2*l2+1, 2*l3+1), dtype=complex)
    for a, m1 in enumerate(range(-l1, l1+1)):
        for b, m2 in enumerate(range(-l2, l2+1)):
            for c, m3 in enumerate(range(-l3, l3+1)):
                C[a, b, c] = _cg(l1, m1, l2, m2, l3, m3)
    T = np.einsum('am,bn,ck,mnk->abc', _umat(l1), _umat(l2), _umat(l3).conj(), C)
    Tr, Ti = np.real(T), np.imag(T)
    T = Tr if np.linalg.norm(Tr) >= np.linalg.norm(Ti) else Ti
    n = np.linalg.norm(T)
    return None if n < 1e-8 else (T / n).astype(np.float32)


PATHS = {l3: [] for l3 in range(LMAX + 1)}
for _l1 in range(LMAX + 1):
    for _l2 in range(LMAX + 1):
        for _l3 in range(abs(_l1 - _l2), min(_l1 + _l2, LMAX) + 1):
            _T = _real_coupling(_l1, _l2, _l3)
            if _T is not None:
                PATHS[_l3].append((_l1, _l2, _T))
NPATH = {l3: len(PATHS[l3]) for l3 in range(LMAX + 1)}

# (p,k)-rows for the l3>=1 part: 152 rows
_PKROWS = []
for _l3 in (1, 2, 3):
    for _p in range(NPATH[_l3]):
        for _kk in range(2*_l3+1):
            _PKROWS.append((_l3, _p, _kk))
NPK1 = len(_PKROWS)                          # 152
assert NPK1 == 152

NCOL = 240                                   # Vn' columns: (k-1)*16 + d


def _build_ttT152():
    """tt[(p,k)-row, (i,j)] for l3>=1 with scale, fan and L2-dot fold baked in."""
    T2 = {}
    for p, (l1, l2, T) in enumerate(PATHS[0]):
        T2[l1] = T[:, :, 0]
    tt = np.zeros((NPK1, 256), np.float64)
    for r, (l3, p, ko) in enumerate(_PKROWS):
        l1, l2, T = PATHS[l3][p]
        i0, j0 = SLSTART[l1], SLSTART[l2]
        scale = math.sqrt(2*l3+1) / math.sqrt(MUL * NPATH[l3])
        acc = np.zeros((2*l1+1, 2*l2+1), np.float64)
        for kk in range(2*l3+1):
            t2 = T2[l3][ko, kk]
            if t2 != 0.0:
                acc += t2 * T[:, :, kk]
        blk = acc * scale
        for ii in range(2*l1+1):
            for jj in range(2*l2+1):
                tt[r, (i0+ii)*16 + (j0+jj)] = blk[ii, jj]
    return tt


def _build_cst(W_v1, W_v2, W_v3):
    """Host C: cst[m] = ttT152.T @ wk[m]  -> [16, 256, 240] float32."""
    tt = _build_ttT152()                     # [152, 256]
    Wv = {1: W_v1.astype(np.float64), 2: W_v2.astype(np.float64),
          3: W_v3.astype(np.float64)}
    wk = np.zeros((MUL, NPK1, NCOL), np.float64)
    for r, (l3, p, ko) in enumerate(_PKROWS):
        kabs = SLSTART[l3] + ko              # 1..15
        for m in range(MUL):
            wk[m, r, (kabs-1)*16 + np.arange(MUL)] = Wv[l3][p*16 + m, :]
    cst = np.einsum('rc,mrn->mcn', tt, wk)   # [16, 256, 240]
    return cst.astype(np.float32)


def _build_t0():
    """T0[(i,m), p*16+m] = diag of the (l,l,0) couplings (exactly diagonal)."""
    t0 = np.zeros((256, 64), np.float32)
    for p, (l1, l2, T) in enumerate(PATHS[0]):
        i0 = SLSTART[l1]
        d = np.diagonal(T[:, :, 0])
        for m in range(MUL):
            for ii in range(2*l1+1):
                t0[(i0 + ii)*16 + m, p*16 + m] = d[ii]
    return t0


def _build_rt():
    """RT[row (k-1)*16+m chunked by 120, half, li*16+m] = [k in SL[l]]."""
    rt = np.zeros((120, 2, 48), np.float32)
    for m in range(MUL):
        for k in range(1, 16):
            r = (k-1)*16 + m
            h, rloc = divmod(r, 120)
            for li, l in enumerate((1, 2, 3)):
                lo, hi = SLICES[l]
                if lo <= k < hi:
                    rt[rloc, h, li*16 + m] = 1.0
    return rt


# ------------------------------------------------------------- device program

_PROG_CACHE = {}


def _build_program(CAP):
    NT = CAP // 128
    CH = [(s, min(512, CAP - s)) for s in range(0, CAP, 512)]
    nc = bacc.Bacc("TRN2", target_bir_lowering=False, debug=False,
                   num_devices=N_CORES)
    D = {}

    def dp(name, shape, dt=F32, out=False):
        D[name] = nc.declare_dram_parameter(name, list(shape), dt, isOutput=out)
        return D[name]

    dp("vec", [128, NT, 3]); dp("maskt", [128, NT])
    dp("ohs", [128, CAP], BF16); dp("ohr", [128, CAP], BF16)
    dp("smat", [128, NT, 256], BF16); dp("gmat", [128, 2, CAP], BF16)
    dp("tabs", [128, 32], BF16); dp("tabr", [128, 32], BF16)
    dp("w1b", [8, 32], BF16)
    dp("wtb2", [32, 64], BF16); dp("wtb3", [64, 128], BF16); dp("wtb4", [128, 256], BF16)
    dp("ww0", [128, 2, 16], BF16); dp("ww1", [128, 2, 16], BF16); dp("ww2", [128, 2, 16], BF16)
    dp("wl11", [128, 2, 256], BF16); dp("wl11t", [64, 256], BF16)
    dp("wl12", [128, 2, 256], BF16); dp("wl13", [128, 2, 256], BF16)
    dp("wl21", [128, 2, 256], BF16); dp("wl21t", [48, 256], BF16)
    dp("wl22", [128, 2, 256], BF16); dp("wl23", [128, 2, 256], BF16)
    dp("wh", [128, 2, 128], BF16); dp("wout", [128, 1], BF16)
    dp("cst", [128, 2, 16, NCOL], BF16)
    dp("t0", [128, 2, 64], BF16)
    dp("rt", [120, 2, 48], BF16)
    dp("repj", [16, 256], BF16); dp("repja", [16, 256], BF16)
    dp("repibig", [128, 16, 128], BF16)
    dp("e16b", [16, 256], BF16)
    dp("iden", [128, 128])
    dp("ones1", [1, 128], BF16); dp("kacol", [128, 1], BF16)
    dp("outv", [1, CAP], out=True)

    S3 = math.sqrt(3.0); S15 = math.sqrt(15.0); S5 = math.sqrt(5.0)
    S358 = math.sqrt(35.0/8.0); S105 = math.sqrt(105.0)
    S218 = math.sqrt(21.0/8.0); S7 = math.sqrt(7.0)

    with tile.TileContext(nc) as tc:
        pst_bufs = 3 if PR_TRANSPOSE else 2
        psr_bufs = 3 if PR_TRANSPOSE else 2
        with tc.tile_pool(name="perm", bufs=1) as perm, \
             tc.tile_pool(name="wpool", bufs=1) as wpool, \
             tc.tile_pool(name="tmp", bufs=2) as tmp, \
             tc.tile_pool(name="chp", bufs=2) as chp, \
             tc.tile_pool(name="hp", bufs=2) as hp, \
             tc.tile_pool(name="pst", bufs=pst_bufs, space="PSUM") as pst, \
             tc.tile_pool(name="psacc", bufs=1, space="PSUM") as psacc, \
             tc.tile_pool(name="psr", bufs=psr_bufs, space="PSUM") as psr:

            # ---- persistent SBUF
            geo = perm.tile([128, NT, 32], F32, tag="geo", name="geo")
            geoT = perm.tile([25, CAP], BF16, tag="geoT", name="geoT")
            esb = perm.tile([128, CAP], BF16, tag="esb", name="esb")
            xsb = perm.tile([128, 2, CAP], BF16, tag="xsb", name="xsb")
            v16 = perm.tile([16, CAP], BF16, tag="v16", name="v16")
            tp0sb = perm.tile([64, CAP], BF16, tag="tp0sb", name="tp0sb")
            tp02 = perm.tile([48, CAP], BF16, tag="tp02", name="tp02")
            vnpP = perm.tile([120, CAP], BF16, tag="vnpP", name="vnpP")
            vnpQ = perm.tile([120, CAP], BF16, tag="vnpQ", name="vnpQ")
            node_nm = perm.tile([128, 2, 256], BF16, tag="node_nm", name="node_nm")
            smatsb = perm.tile([128, NT, 256], BF16, tag="smatsb", name="smatsb")
            nc.sync.dma_start(smatsb[:], D["smat"][:])

            # ---- geometry inputs first (unblock geometry before weight DMAs)
            vec = perm.tile([128, NT, 3], F32, tag="vec", name="vec")
            nc.sync.dma_start(vec[:], D["vec"][:])
            def ka(anchor, np_, nf):
                """keep-alive: standalone ldweights anchored on a recent tile
                (keeps the PE HAM activity window busy, no PSUM needed)."""
                nc.tensor.ldweights(anchor)


            W = {}
            W["iden"] = wpool.tile([128, 128], F32, tag="w_iden", name="w_iden")
            nc.sync.dma_start(W["iden"][:], D["iden"][:])
            kaw = pst.tile([1, 128], F32, tag="ps", name="kaw")
            for _ in range(60):
                nc.tensor.matmul(kaw[:], W["iden"][:, 0:1], W["iden"][:],
                                 start=True, stop=True)

            prime = tmp.tile([1, 4], F32, tag="prime", name="prime")
            nc.vector.memset(prime[:], 0.25)
            nc.scalar.activation(prime[:], prime[:], AF.Sqrt)
            nc.scalar.activation(prime[:], prime[:], AF.Sin)
            nc.scalar.activation(prime[:], prime[:], AF.Silu)
            mask = tmp.tile([128, NT], F32, tag="mask", name="mask")
            nc.sync.dma_start(mask[:], D["maskt"][:])

            # ---- weights in SBUF
            for nm, shape, dt in [
                ("tabs", [128, 32], BF16), ("tabr", [128, 32], BF16),
                ("w1b", [8, 32], BF16),
                ("wtb2", [32, 64], BF16), ("wtb3", [64, 128], BF16),
                ("wtb4", [128, 256], BF16),
                ("ww0", [128, 2, 16], BF16), ("ww1", [128, 2, 16], BF16),
                ("ww2", [128, 2, 16], BF16),
                ("wl11", [128, 2, 256], BF16), ("wl11t", [64, 256], BF16),
                ("wl12", [128, 2, 256], BF16), ("wl13", [128, 2, 256], BF16),
                ("wl21", [128, 2, 256], BF16), ("wl21t", [48, 256], BF16),
                ("wl22", [128, 2, 256], BF16), ("wl23", [128, 2, 256], BF16),
                ("wh", [128, 2, 128], BF16), ("wout", [128, 1], BF16),
                ("t0", [128, 2, 64], BF16), ("rt", [120, 2, 48], BF16),
                ("repj", [16, 256], BF16), ("repja", [16, 256], BF16),
                ("repibig", [128, 16, 128], BF16),
                ("e16b", [16, 256], BF16),
                ("ones1", [1, 128], BF16), ("kacol", [128, 1], BF16),
                ("cst", [128, 2, 16, NCOL], BF16),
            ]:
                W[nm] = wpool.tile(shape, dt, tag="w_" + nm, name="w_" + nm)
                nc.sync.dma_start(W[nm][:], D[nm][:])

            # ================= geometry (edge-major)
            u = perm.tile([128, NT, 3], F32, tag="u", name="u")

            def t2(tag):
                return tmp.tile([128, NT], F32, tag=tag, name=tag)

            vv = tmp.tile([128, NT, 3], F32, tag="vv", name="vv")
            nc.vector.tensor_tensor(vv[:], vec[:], vec[:], op=AL.mult)
            d2 = t2("d2")
            nc.vector.tensor_reduce(d2[:], vv[:], axis=mybir.AxisListType.X, op=AL.add)
            d = t2("d")
            nc.scalar.activation(d[:], d2[:], AF.Sqrt)
            rec = t2("rec")
            nc.vector.reciprocal(rec[:], d[:])
            nc.vector.tensor_tensor(
                u[:], vec[:], rec[:, :, None].broadcast_to((128, NT, 3)), op=AL.mult)
            # envelope (p=6): 1 + d^6*(-28 + 48d - 21 d^2), then mask
            d3 = t2("d3"); d6 = t2("d6"); q = t2("q"); env = t2("env")
            nc.vector.tensor_tensor(d3[:], d2[:], d[:], op=AL.mult)
            nc.vector.tensor_tensor(d6[:], d3[:], d3[:], op=AL.mult)
            ts1 = t2("ts1")
            nc.vector.tensor_scalar(ts1[:], d[:], 48.0, None, op0=AL.mult)
            nc.vector.scalar_tensor_tensor(q[:], d2[:], -21.0, ts1[:],
                                           op0=AL.mult, op1=AL.add)
            nc.vector.tensor_scalar(q[:], q[:], -28.0, None, op0=AL.add)
            nc.vector.tensor_tensor(env[:], d6[:], q[:], op=AL.mult)
            nc.vector.tensor_scalar(env[:], env[:], 1.0, None, op0=AL.add)
            nc.vector.tensor_tensor(env[:], env[:], mask[:], op=AL.mult)
            # sines via recurrence: s1=sin(pi d), c=sin(pi d + pi/2)
            nc.scalar.activation(geo[:, :, 16], d[:], AF.Sin, scale=math.pi)
            c1 = t2("c1")
            hpi = tmp.tile([128, 1], F32, tag="hpi", name="hpi")
            nc.vector.memset(hpi[:], math.pi / 2.0)
            nc.scalar.activation(c1[:], d[:], AF.Sin, scale=-math.pi,
                                 bias=hpi[:])
            nc.vector.tensor_scalar(c1[:], c1[:], 2.0, None, op0=AL.mult)
            nc.vector.tensor_tensor(geo[:, :, 17], c1[:], geo[:, :, 16], op=AL.mult)
            for n in range(3, 9):
                sn = t2("sn")
                nc.vector.tensor_tensor(sn[:], c1[:], geo[:, :, 14+n], op=AL.mult)
                nc.vector.tensor_tensor(geo[:, :, 15+n], sn[:], geo[:, :, 13+n],
                                        op=AL.subtract)
            renv = t2("renv")
            nc.vector.tensor_tensor(renv[:], env[:], rec[:], op=AL.mult)
            nc.vector.tensor_scalar(renv[:], renv[:], math.sqrt(2.0), None, op0=AL.mult)
            nc.vector.tensor_tensor(
                geo[:, :, 16:24], geo[:, :, 16:24],
                renv[:, :, None].broadcast_to((128, NT, 8)), op=AL.mult)
            # spherical harmonics into geo cols 0..15
            ux, uy, uz = u[:, :, 0], u[:, :, 1], u[:, :, 2]
            nc.vector.memset(geo[:, :, 0], 1.0)
            nc.vector.tensor_scalar(geo[:, :, 1], uy, S3, None, op0=AL.mult)
            nc.vector.tensor_scalar(geo[:, :, 2], uz, S3, None, op0=AL.mult)
            nc.vector.tensor_scalar(geo[:, :, 3], ux, S3, None, op0=AL.mult)
            xy = t2("xy"); yz = t2("yz"); xz = t2("xz")
            x2 = t2("x2"); y2 = t2("y2"); z2 = t2("z2"); xmy = t2("xmy")
            nc.vector.tensor_tensor(xy[:], ux, uy, op=AL.mult)
            nc.vector.tensor_tensor(yz[:], uy, uz, op=AL.mult)
            nc.vector.tensor_tensor(xz[:], ux, uz, op=AL.mult)
            nc.vector.tensor_tensor(x2[:], ux, ux, op=AL.mult)
            nc.vector.tensor_tensor(y2[:], uy, uy, op=AL.mult)
            nc.vector.tensor_tensor(z2[:], uz, uz, op=AL.mult)
            nc.vector.tensor_tensor(xmy[:], x2[:], y2[:], op=AL.subtract)
            nc.vector.tensor_scalar(geo[:, :, 4], xy[:], S15, None, op0=AL.mult)
            nc.vector.tensor_scalar(geo[:, :, 5], yz[:], S15, None, op0=AL.mult)
            nc.vector.tensor_scalar(geo[:, :, 6], z2[:], 1.5*S5, 0.5*S5,
                                    op0=AL.mult, op1=AL.subtract)
            nc.vector.tensor_scalar(geo[:, :, 7], xz[:], S15, None, op0=AL.mult)
            nc.vector.tensor_scalar(geo[:, :, 8], xmy[:], 0.5*S15, None, op0=AL.mult)
            g1 = t2("g1")
            nc.vector.scalar_tensor_tensor(g1[:], x2[:], 3.0, y2[:],
                                           op0=AL.mult, op1=AL.subtract)
            nc.vector.tensor_tensor(g1[:], g1[:], uy, op=AL.mult)
            nc.vector.tensor_scalar(geo[:, :, 9], g1[:], S358, None, op0=AL.mult)
            g2 = t2("g2")
            nc.vector.tensor_tensor(g2[:], xy[:], uz, op=AL.mult)
            nc.vector.tensor_scalar(geo[:, :, 10], g2[:], S105, None, op0=AL.mult)
            fz = t2("fz")
            nc.vector.tensor_scalar(fz[:], z2[:], 5.0, 1.0, op0=AL.mult,
                                    op1=AL.subtract)
            g3 = t2("g3")
            nc.vector.tensor_tensor(g3[:], fz[:], uy, op=AL.mult)
            nc.vector.tensor_scalar(geo[:, :, 11], g3[:], S218, None, op0=AL.mult)
            f2 = t2("f2")
            nc.vector.tensor_scalar(f2[:], fz[:], -2.0, None, op0=AL.add)
            g4 = t2("g4")
            nc.vector.tensor_tensor(g4[:], f2[:], uz, op=AL.mult)
            nc.vector.tensor_scalar(geo[:, :, 12], g4[:], 0.5*S7, None, op0=AL.mult)
            g5 = t2("g5")
            nc.vector.tensor_tensor(g5[:], fz[:], ux, op=AL.mult)
            nc.vector.tensor_scalar(geo[:, :, 13], g5[:], S218, None, op0=AL.mult)
            g6 = t2("g6")
            nc.vector.tensor_tensor(g6[:], xmy[:], uz, op=AL.mult)
            nc.vector.tensor_scalar(geo[:, :, 14], g6[:], 0.5*S105, None, op0=AL.mult)
            g7 = t2("g7")
            nc.vector.tensor_tensor(g7[:], xmy[:], ux, op=AL.mult)
            nc.vector.tensor_scalar(geo[:, :, 15], g7[:], S358, None, op0=AL.mult)
            nc.vector.tensor_copy(geo[:, :, 24], env[:])
            # per-tile fused transpose -> geoT; bessel/env split via SBUF DMA
            for t in range(NT):
                tc_ = slice(t*128, (t+1)*128)
                psy = psr.tile([25, 128], F32, tag="pr", name="psy")
                nc.tensor.transpose(psy[:], geo[:, t, 0:25], W["iden"][:])
                nc.vector.tensor_copy(geoT[:, tc_], psy[:])
                ka(geoT[0:16, t*128:t*128+64], 16, 64)


            # ================= two-body MLP -> xsb; esb built here
            for (c0, cn) in CH:
                bbfc = chp.tile([8, 512], BF16, tag="bbfc", name="bbfc")
                nc.gpsimd.dma_start(bbfc[:, 0:cn], geoT[16:24, c0:c0+cn])
                envc = chp.tile([1, 512], BF16, tag="envc", name="envc")
                nc.gpsimd.dma_start(envc[:, 0:cn], geoT[24:25, c0:c0+cn])
                psA = pst.tile([32, 512], F32, tag="ps", name="psA")
                nc.tensor.matmul(psA[:, 0:cn], W["w1b"][:], bbfc[:, 0:cn],
                                 start=True, stop=False)
                oht = chp.tile([128, 512], BF16, tag="b512", name="oht")
                nc.gpsimd.dma_start(oht[:, 0:cn], D["ohs"][:, c0:c0+cn])
                nc.tensor.matmul(psA[:, 0:cn], W["tabs"][:], oht[:, 0:cn],
                                 start=False, stop=False)
                oht2 = chp.tile([128, 512], BF16, tag="b512b", name="oht2")
                nc.gpsimd.dma_start(oht2[:, 0:cn], D["ohr"][:, c0:c0+cn])
                nc.tensor.matmul(psA[:, 0:cn], W["tabr"][:], oht2[:, 0:cn],
                                 start=False, stop=True)
                x1 = chp.tile([32, 512], BF16, tag="f512", name="x1")
                nc.scalar.activation(x1[:, 0:cn], psA[:, 0:cn], AF.Silu)
                psB = psacc.tile([64, 512], F32, tag="pout0", name="psB")
                nc.tensor.matmul(psB[:, 0:cn], W["wtb2"][:], x1[:, 0:cn],
                                 start=True, stop=True)
                x2c = chp.tile([64, 512], BF16, tag="f512b", name="x2c")
                nc.scalar.activation(x2c[:, 0:cn], psB[:, 0:cn], AF.Silu)
                psC = psacc.tile([128, 512], F32, tag="pout1", name="psC")
                nc.tensor.matmul(psC[:, 0:cn], W["wtb3"][:], x2c[:, 0:cn],
                                 start=True, stop=True)
                x3 = chp.tile([128, 512], BF16, tag="f512c", name="x3")
                nc.scalar.activation(x3[:, 0:cn], psC[:, 0:cn], AF.Silu)
                pe = pst.tile([128, 512], F32, tag="ps", name="pe")
                nc.tensor.matmul(pe[:, 0:cn], W["ones1"][:], envc[:, 0:cn],
                                 start=True, stop=True)
                nc.vector.tensor_copy(esb[:, c0:c0+cn], pe[:, 0:cn])
                x3e = chp.tile([128, 512], BF16, tag="f512d", name="x3e")
                nc.vector.tensor_tensor(x3e[:, 0:cn], x3[:, 0:cn],
                                        esb[:, c0:c0+cn], op=AL.mult)
                for oc in range(2):
                    psD = psr.tile([128, 512], F32, tag="pr", name="psD")
                    nc.tensor.matmul(psD[:, 0:cn], W["wtb4"][:, oc*128:(oc+1)*128],
                                     x3e[:, 0:cn], start=True, stop=True)
                    if oc == 0:
                        nc.vector.tensor_copy(xsb[:, oc, c0:c0+cn], psD[:, 0:cn])
                    else:
                        nc.scalar.copy(xsb[:, oc, c0:c0+cn], psD[:, 0:cn])
                ka(xsb[:, 0, c0:c0+64], 128, 64)
                psv = pst.tile([16, 512], F32, tag="ps", name="psv")
                nc.tensor.matmul(psv[0:16, 0:cn], W["ww0"][:, 0, :],
                                 xsb[:, 0, c0:c0+cn], start=True, stop=False)
                nc.tensor.matmul(psv[0:16, 0:cn], W["ww0"][:, 1, :],
                                 xsb[:, 1, c0:c0+cn], start=False, stop=True)
                nc.scalar.copy(v16[:, c0:c0+cn], psv[0:16, 0:cn])

            # ================= shared per-layer pieces
            def scatter_layer(wname, node_dst):
                """w=x@W per tile, M=w(x)Y, node += M^T S; node_dst [128,2,256] bf16."""
                nps = [psacc.tile([128, 256], F32, tag=f"pout{oc}",
                                  name=f"nps{oc}_{wname}") for oc in range(2)]
                for t in range(NT):
                    wps = psr.tile([128, 16], F32, tag="pr", name="wps")
                    nc.tensor.matmul(wps[:], xsb[:, 0, t*128:(t+1)*128],
                                     W[wname][:, 0, :], start=True, stop=False)
                    nc.tensor.matmul(wps[:], xsb[:, 1, t*128:(t+1)*128],
                                     W[wname][:, 1, :], start=False, stop=True)
                    mbf = chp.tile([128, 16, 16], BF16, tag="mbf", name="mbf")
                    nc.vector.tensor_tensor(
                        mbf[:], geo[:, t, 0:16, None].broadcast_to((128, 16, 16)),
                        wps[:, None, :].broadcast_to((128, 16, 16)), op=AL.mult)
                    ka(mbf[:, 0, 0:16], 128, 16)
                    mview = mbf[:].rearrange("p a b -> p (a b)")
                    for oc in range(2):
                        nc.tensor.matmul(nps[oc][:], mview[:, oc*128:(oc+1)*128],
                                         smatsb[:, t, :], start=(t == 0),
                                         stop=(t == NT-1))
                nsb = tmp.tile([128, 2, 256], F32, tag="nsb", name="nsb")
                for oc in range(2):
                    nc.vector.tensor_copy(nsb[:, oc, :], nps[oc][:])
                for oc in range(2):
                    for nk in range(2):
                        pstr = pst.tile([128, 128], F32, tag="ps", name="pstr")
                        nc.tensor.transpose(pstr[:], nsb[:, oc, nk*128:(nk+1)*128],
                                            W["iden"][:])
                        nc.vector.tensor_copy(node_dst[:, nk, oc*128:(oc+1)*128],
                                              pstr[:])

            # ================= layer 1: scatter then TP
            scatter_layer("ww1", node_nm)

            for (c0, cn) in CH:
                # yrep broadcast (one-hot repj): [128,2,cn] Y[j] at rows (i,j)
                yrepsb = chp.tile([128, 2, 512], BF16, tag="yrepsb", name="yrepsb")
                vsb = chp.tile([128, 2, 512], BF16, tag="vsb", name="vsb")
                if PR_TRANSPOSE:
                    yrepps = psr.tile([128, 2, 512], BF16, tag="prb",
                                      name=f"yrep_{c0}")
                    for oc in range(2):
                        nc.tensor.matmul(yrepps[:, oc, 0:cn],
                                         W["repj"][:, oc*128:(oc+1)*128],
                                         geoT[0:16, c0:c0+cn], is_transpose=True,
                                         start=True, stop=True)
                    nc.scalar.copy(yrepsb[:, :, 0:cn], yrepps[:, :, 0:cn])
                    pvb = psr.tile([128, 2, 512], BF16, tag="prb",
                                   name=f"pv_{c0}")
                    for oc in range(2):
                        nc.tensor.matmul(pvb[:, oc, 0:cn],
                                         W["e16b"][:, oc*128:(oc+1)*128],
                                         v16[:, c0:c0+cn], is_transpose=True,
                                         start=True, stop=True)
                    nc.scalar.copy(vsb[:, :, 0:cn], pvb[:, :, 0:cn])
                else:
                    for oc in range(2):
                        yrepps = pst.tile([128, 512], F32, tag="ps",
                                          name=f"yrep{oc}_{c0}")
                        nc.tensor.matmul(yrepps[:, 0:cn],
                                         W["repj"][:, oc*128:(oc+1)*128],
                                         geoT[0:16, c0:c0+cn], start=True, stop=True)
                        nc.scalar.copy(yrepsb[:, oc, 0:cn], yrepps[:, 0:cn])
                    for oc in range(2):
                        pvps = pst.tile([128, 512], F32, tag="ps", name="pvps")
                        nc.tensor.matmul(pvps[:, 0:cn],
                                         W["e16b"][:, oc*128:(oc+1)*128],
                                         v16[:, c0:c0+cn], start=True, stop=True)
                        nc.scalar.copy(vsb[:, oc, 0:cn], pvps[:, 0:cn])
                # gather wY: node_nm^T @ gmat  (rows (m,i))
                gch = chp.tile([128, 2, 512], BF16, tag="gch", name="gch")
                nc.sync.dma_start(gch[:, :, 0:cn], D["gmat"][:, :, c0:c0+cn])
                wyv = chp.tile([128, 2, 512], BF16, tag="wyv", name="wyv")
                for oc in range(2):
                    pwps = pst.tile([128, 512], F32, tag="ps", name=f"pw{oc}_{c0}")
                    for kc in range(2):
                        nc.tensor.matmul(pwps[:, 0:cn],
                                         node_nm[:, kc, oc*128:(oc+1)*128],
                                         gch[:, kc, 0:cn], start=(kc == 0),
                                         stop=(kc == 1))
                    nc.vector.tensor_tensor(wyv[:, oc, 0:cn], pwps[:, 0:cn],
                                            vsb[:, oc, 0:cn], op=AL.mult)
                # tp0: wyv16 = wyv * yrepI (rows (i,m): Y[i]); contract T0
                yrepi = chp.tile([128, 2, 512], BF16, tag="yrepi", name="yrepi")
                for oc in range(2):
                    pyi = pst.tile([128, 512], F32, tag="ps", name="pyi")
                    nc.tensor.matmul(pyi[:, 0:cn],
                                     W["repja"][:, oc*128:(oc+1)*128],
                                     geoT[0:16, c0:c0+cn], start=True, stop=True)
                    nc.scalar.copy(yrepi[:, oc, 0:cn], pyi[:, 0:cn])
                wyv16 = chp.tile([128, 2, 512], BF16, tag="wyv16", name="wyv16")
                nc.vector.tensor_tensor(wyv16[:, :, 0:cn], wyv[:, :, 0:cn],
                                        yrepi[:, :, 0:cn], op=AL.mult)
                ptp0 = pst.tile([64, 512], F32, tag="ps", name="ptp0")
                for kc in range(2):
                    nc.tensor.matmul(ptp0[:, 0:cn], W["t0"][:, kc, :],
                                     wyv16[:, kc, 0:cn], start=(kc == 0),
                                     stop=(kc == 1))
                nc.vector.tensor_copy(tp0sb[:, c0:c0+cn], ptp0[:, 0:cn])

                # m-loop: pr broadcast -> product -> cst contraction
                pouts = [psacc.tile([120, 512], F32, tag=f"pout{i}",
                                    name=f"pout{i}_{c0}") for i in range(2)]

                def make_pbf(m, c0=c0, cn=cn, wyv=wyv, yrepsb=yrepsb):
                    if PR_TRANSPOSE:
                        pr = psr.tile([128, 2, 512], BF16, tag="prb",
                                      name=f"pr{m}_{c0}")
                        for oc in range(2):
                            nc.tensor.matmul(pr[:, oc, 0:cn],
                                             W["repibig"][:, m % 8,
                                                          oc*128:(oc+1)*128],
                                             wyv[:, m // 8, 0:cn],
                                             is_transpose=True,
                                             start=True, stop=True)
                    else:
                        pr = psr.tile([128, 2, 512], F32, tag="pr",
                                      name=f"pr{m}_{c0}")
                        for oc in range(2):
                            nc.tensor.matmul(pr[:, oc, 0:cn],
                                             W["repibig"][:, m, :],
                                             wyv[:, oc, 0:cn],
                                             start=True, stop=True)
                    pbf = chp.tile([128, 2, 512], BF16, tag="pbf",
                                   name=f"pbf{m}_{c0}")
                    if m < ACT_EVAC:
                        prs = chp.tile([128, 2, 512], BF16, tag="prs",
                                       name=f"prs{m}_{c0}")
                        nc.scalar.copy(prs[:, :, 0:cn], pr[:, :, 0:cn])
                        nc.vector.tensor_tensor(pbf[:, :, 0:cn], prs[:, :, 0:cn],
                                                yrepsb[:, :, 0:cn], op=AL.mult)
                    else:
                        nc.vector.tensor_tensor(pbf[:, :, 0:cn], pr[:, :, 0:cn],
                                                yrepsb[:, :, 0:cn], op=AL.mult)
                    return pbf

                pending = make_pbf(0)
                for m in range(MUL):
                    nxt = make_pbf(m + 1) if m + 1 < MUL else None
                    for kc in range(2):
                        for o3 in range(2):
                            nc.tensor.matmul(
                                pouts[o3][:, 0:cn],
                                W["cst"][:, kc, m, o3*120:(o3+1)*120],
                                pending[:, kc, 0:cn],
                                start=(m == 0 and kc == 0),
                                stop=(m == MUL-1 and kc == 1))
                    pending = nxt
                nc.vector.tensor_copy(vnpP[:, c0:c0+cn], pouts[0][:, 0:cn])
                nc.vector.tensor_copy(vnpQ[:, c0:c0+cn], pouts[1][:, 0:cn])

            # ================= MLP block (shared for both layers)
            def mlp(wl_a, wl_t, t_extra, wl_b, wl_c, tk):
                for (c0, cn) in CH:
                    h1 = hp.tile([128, 2, 512], BF16, tag="h1", name="h1")
                    for oc in range(2):
                        ph = pst.tile([128, 512], F32, tag="ps", name="ph")
                        ocs = slice(oc*128, (oc+1)*128)
                        nc.tensor.matmul(ph[:, 0:cn], W[wl_a][:, 0, ocs],
                                         xsb[:, 0, c0:c0+cn], start=True, stop=False)
                        nc.tensor.matmul(ph[:, 0:cn], W[wl_a][:, 1, ocs],
                                         xsb[:, 1, c0:c0+cn], start=False, stop=False)
                        nc.tensor.matmul(ph[:, 0:cn], W[wl_t][:, ocs],
                                         t_extra[0:tk, c0:c0+cn], start=False, stop=True)
                        nc.scalar.activation(h1[:, oc, 0:cn], ph[:, 0:cn], AF.Silu)
                    h2 = hp.tile([128, 2, 512], BF16, tag="h2", name="h2")
                    for oc in range(2):
                        ph2 = psr.tile([128, 512], F32, tag="pr", name="ph2")
                        ocs = slice(oc*128, (oc+1)*128)
                        for kc in range(2):
                            nc.tensor.matmul(ph2[:, 0:cn], W[wl_b][:, kc, ocs],
                                             h1[:, kc, 0:cn], start=(kc == 0),
                                             stop=(kc == 1))
                        nc.scalar.activation(h2[:, oc, 0:cn], ph2[:, 0:cn], AF.Silu)
                    nc.vector.tensor_tensor(h2[:, :, 0:cn], h2[:, :, 0:cn],
                                            esb[:, None, c0:c0+cn].broadcast_to(
                                                (128, 2, cn)), op=AL.mult)
                    ka(h2[:, 0, 0:64], 128, 64)
                    for oc in range(2):
                        ph3 = psacc.tile([128, 512], F32, tag=f"pout{oc}",
                                         name="ph3")
                        ocs = slice(oc*128, (oc+1)*128)
                        for kc in range(2):
                            nc.tensor.matmul(ph3[:, 0:cn], W[wl_c][:, kc, ocs],
                                             h2[:, kc, 0:cn], start=(kc == 0),
                                             stop=(kc == 1))
                        nc.vector.tensor_tensor(
                            xsb[:, oc, c0:c0+cn], xsb[:, oc, c0:c0+cn],
                            ph3[:, 0:cn], op=AL.add)

            mlp("wl11", "wl11t", tp0sb, "wl12", "wl13", 64)

            # ================= layer 2: scatter + feature-major dots
            node2 = perm.tile([128, 2, 256], BF16, tag="node2", name="node2")
            scatter_layer("ww2", node2)
            for (c0, cn) in CH:
                gch2 = chp.tile([128, 2, 512], BF16, tag="gch", name="gch2")
                nc.sync.dma_start(gch2[:, :, 0:cn], D["gmat"][:, :, c0:c0+cn])
                prods = []
                for h, vnph in ((0, vnpP), (1, vnpQ)):
                    pwh = psacc.tile([120, 512], F32, tag=f"pout{h}", name=f"pwh{h}")
                    for kc in range(2):
                        nc.tensor.matmul(pwh[:, 0:cn],
                                         node2[:, kc, 16+h*120:16+(h+1)*120],
                                         gch2[:, kc, 0:cn], start=(kc == 0),
                                         stop=(kc == 1))
                    prodh = chp.tile([120, 512], BF16, tag=f"prod{h}",
                                     name=f"prod{h}_{c0}")
                    nc.vector.tensor_tensor(prodh[:, 0:cn], pwh[:, 0:cn],
                                            vnph[:, c0:c0+cn], op=AL.mult)
                    prods.append(prodh)
                ptp2 = pst.tile([48, 512], F32, tag="ps", name="ptp2")
                for h in range(2):
                    nc.tensor.matmul(ptp2[:, 0:cn], W["rt"][:, h, :],
                                     prods[h][:, 0:cn], start=(h == 0),
                                     stop=(h == 1))
                nc.vector.tensor_copy(tp02[:, c0:c0+cn], ptp2[:, 0:cn])
                ka(tp02[:, c0:c0+64], 48, 64)

            mlp("wl21", "wl21t", tp02, "wl22", "wl23", 48)

            # ================= head
            for ci, (c0, cn) in enumerate(CH):
                psh = psacc.tile([128, 512], F32, tag=f"pout{ci % 2}", name="psh")
                for kc in range(2):
                    nc.tensor.matmul(psh[:, 0:cn], W["wh"][:, kc, :],
                                     xsb[:, kc, c0:c0+cn], start=(kc == 0),
                                     stop=(kc == 1))
                xh = chp.tile([128, 512], BF16, tag="f512", name="xh")
                nc.vector.tensor_tensor(xh[:, 0:cn], psh[:, 0:cn],
                                        esb[:, c0:c0+cn], op=AL.mult)
                ka(xh[:, 0:64], 128, 64)
                pso = pst.tile([1, 512], F32, tag="ps", name="pso")
                nc.tensor.matmul(pso[:, 0:cn], W["wout"][:], xh[:, 0:cn],
                                 start=True, stop=True)
                osb = chp.tile([1, 512], F32, tag="f512", name="osb")
                nc.vector.tensor_copy(osb[:, 0:cn], pso[:, 0:cn])
                nc.sync.dma_start(D["outv"][:, c0:c0+cn], osb[:, 0:cn])

    nc.compile()
    return nc


# ---------------------------------------------------------------- host side


def _to_em(a, NT):
    """[CAP, ...] -> [128, NT, ...] edge-major (edge = t*128+p -> row p col t)."""
    return np.ascontiguousarray(a.reshape(NT, 128, *a.shape[1:]).swapaxes(0, 1))


def _prep_inputs(inputs):
    inputs = {k: np.asarray(v) for k, v in inputs.items()}
    senders = inputs["senders"].astype(np.int64)
    receivers = inputs["receivers"].astype(np.int64)
    species = inputs["species"].astype(np.int64)
    vectors = inputs["vectors"].astype(np.float32)
    eps = 1.0 / math.sqrt(1.0 + float(inputs["varepsilon"])**2)
    a2 = float(inputs["alpha"])**2

    core_of = senders // NPC
    idxs = [np.nonzero(core_of == c)[0] for c in range(N_CORES)]
    maxk = max(len(i) for i in idxs)
    CAP = ((maxk + 127) // 128) * 128
    NT = CAP // 128

    sc = 1.0 / math.sqrt(N_RBF + 2*EMB)
    emb = inputs["emb"].astype(np.float64)
    tabS = np.zeros((128, 32), np.float64)
    tabR = np.zeros((128, 32), np.float64)
    tabS[:100] = emb @ (inputs["W_tb1"][N_RBF:N_RBF+EMB].astype(np.float64) * sc)
    tabR[:100] = emb @ (inputs["W_tb1"][N_RBF+EMB:].astype(np.float64) * sc)

    shared = {
        "w1b": (inputs["W_tb1"][:N_RBF] * sc).astype(BF),
        "tabs": tabS.astype(BF), "tabr": tabR.astype(BF),
        "wtb2": (inputs["W_tb2"] / math.sqrt(32)).astype(BF),
        "wtb3": (inputs["W_tb3"] / math.sqrt(64)).astype(BF),
        "wtb4": (inputs["W_tb4"] / math.sqrt(128)).astype(BF),
        "ww0": ((inputs["W_w0"] / math.sqrt(HIDDEN)).reshape(2, 128, 16).swapaxes(0, 1)).astype(BF),
        "ww1": ((inputs["W_w1"] * eps / math.sqrt(HIDDEN)).reshape(2, 128, 16).swapaxes(0, 1)).astype(BF),
        "ww2": ((inputs["W_w2"] * eps * 0.5 / math.sqrt(HIDDEN)).reshape(2, 128, 16).swapaxes(0, 1)).astype(BF),
        "wl12": ((inputs["W_l12"] / math.sqrt(HIDDEN)).reshape(2, 128, 256).swapaxes(0, 1)).astype(BF),
        "wl13": ((inputs["W_l13"] / math.sqrt(HIDDEN) * a2 / (1 + a2) * 2.0).reshape(2, 128, 256).swapaxes(0, 1)).astype(BF),
        "wl22": ((inputs["W_l22"] / math.sqrt(HIDDEN)).reshape(2, 128, 256).swapaxes(0, 1)).astype(BF),
        "wl23": ((inputs["W_l23"] / math.sqrt(HIDDEN) * a2 / (1 + a2) * 4.0).reshape(2, 128, 256).swapaxes(0, 1)).astype(BF),
        "wh": ((inputs["W_h"] * 0.25 / math.sqrt(HIDDEN)).reshape(2, 128, 128).swapaxes(0, 1)).astype(BF),
        "wout": (inputs["W_out"] / math.sqrt(128)).astype(BF),
    }
    s320 = 1.0 / math.sqrt(320)
    wl11 = inputs["W_l11"] * s320
    shared["wl11"] = wl11[:256].reshape(2, 128, 256).swapaxes(0, 1).astype(BF)
    shared["wl11t"] = wl11[256:320].astype(BF)
    wl21 = inputs["W_l21"] * s320
    wl21 = wl21.copy(); wl21[:256] *= 0.5
    shared["wl21"] = wl21[:256].reshape(2, 128, 256).swapaxes(0, 1).astype(BF)
    shared["wl21t"] = wl21[256+16:320].astype(BF)     # drop zero p0 block

    cstm = _build_cst(inputs["W_v1"], inputs["W_v2"], inputs["W_v3"])
    # cst[p, kc, m, c] = cstm[m][kc*128+p, c]
    cst = np.ascontiguousarray(
        cstm.reshape(MUL, 2, 128, NCOL).transpose(2, 1, 0, 3))
    shared["cst"] = cst.astype(BF)
    t0full = _build_t0()                         # [256, 64]
    shared["t0"] = np.ascontiguousarray(
        t0full.reshape(2, 128, 64).swapaxes(0, 1)).astype(BF)
    shared["rt"] = _build_rt().astype(BF)

    repj = np.zeros((16, 256), np.float32)
    for i in range(16):
        for j in range(16):
            repj[j, i*16+j] = 1.0
    repibig = np.zeros((128, 16, 128), np.float32)
    for il in range(8):
        for mm in range(16):
            for j in range(16):
                repibig[il*16+mm, mm, il*16+j] = 1.0
    e16b = np.zeros((16, 256), np.float32)
    for m in range(16):
        for i in range(16):
            e16b[m, i*16+m] = 1.0
    repjA = np.zeros((16, 256), np.float32)
    for i in range(16):
        for mm in range(16):
            repjA[i, i*16+mm] = 1.0
    shared["repja"] = repjA.astype(BF)
    shared["repj"] = repj.astype(BF)
    shared["repibig"] = repibig.astype(BF)
    shared["e16b"] = e16b.astype(BF)
    shared["iden"] = np.eye(128, dtype=np.float32)
    shared["ones1"] = np.ones((1, 128), np.float32).astype(BF)
    shared["kacol"] = np.ones((128, 1), np.float32).astype(BF)

    for k in list(shared):
        if shared[k].dtype not in (np.dtype(np.float32), np.dtype(BF)):
            shared[k] = shared[k].astype(np.float32)
        shared[k] = np.ascontiguousarray(shared[k])

    in_maps = []
    for c in range(N_CORES):
        idx = idxs[c]
        k = len(idx)
        vec = np.zeros((CAP, 3), np.float32); vec[:, 2] = 0.5
        vec[:k] = vectors[idx]
        maskv = np.zeros(CAP, np.float32); maskv[:k] = 1.0
        sl = np.zeros(CAP, np.int64)
        sl[:k] = senders[idx] - c * NPC
        spe_s = np.full(CAP, 127, np.int64); spe_s[:k] = species[senders[idx]]
        spe_r = np.full(CAP, 127, np.int64); spe_r[:k] = species[receivers[idx]]
        ohs = np.zeros((128, CAP), np.float32)
        ohs[spe_s, np.arange(CAP)] = 1.0
        ohr = np.zeros((128, CAP), np.float32)
        ohr[spe_r, np.arange(CAP)] = 1.0
        smat = np.zeros((CAP, 256), np.float32)
        smat[np.arange(k), sl[:k]] = 1.0
        gmat = np.zeros((2, 128, CAP), np.float32)
        gmat[sl[:k] // 128, sl[:k] % 128, np.arange(k)] = 1.0
        m = dict(shared)
        m["vec"] = _to_em(vec, CAP // 128)
        m["maskt"] = _to_em(maskv, CAP // 128)
        m["ohs"] = ohs.astype(BF)
        m["ohr"] = ohr.astype(BF)
        m["smat"] = np.ascontiguousarray(
            smat.reshape(CAP // 128, 128, 256).swapaxes(0, 1)).astype(BF)
        m["gmat"] = np.ascontiguousarray(gmat.swapaxes(0, 1)).astype(BF)
        m = {k2: np.ascontiguousarray(v) for k2, v in m.items()}
        in_maps.append(m)
    return in_maps, idxs, CAP


def _run(inputs, trace=False, tmpdir=None):
    in_maps, idxs, CAP = _prep_inputs(inputs)
    if CAP not in _PROG_CACHE:
        _PROG_CACHE[CAP] = _build_program(CAP)
    nc = _PROG_CACHE[CAP]
    res = run_bass_kernel_spmd(nc, in_maps, list(range(N_CORES)), trace=trace,
                               tmpdir=tmpdir)
    out = np.zeros((E, 1), np.float32)
    for c in range(N_CORES):
        k = len(idxs[c])
        out[idxs[c], 0] = res.results[c]["outv"][0, :k]
    return out, res


def kernel(**inputs):
    out, _ = _run(inputs, trace=False)
    return out

